# revision 9
# baseline (speedup 1.0000x reference)
"""GQA attention forward (dense_transformer) on 8 TRN2 NeuronCores.

Problem: x[2,2048,2048] -> RoPE'd GQA attention (16 q-heads, 4 kv-heads,
head_dim 128, causal) -> out @ Wo, f32.

Sharding: core = (batch b, kv-head g). Each core handles one batch and one
kv-group (4 q-heads + its kv head): computes q/k/v projections for its
columns of Wq/Wk/Wv, attention for its 4 heads, and a partial output
through its 512 rows of Wo. Host sums the 4 partials per batch.

On-device layout tricks (all decided at host level):
 - x is transposed on host (xT [D, S]) so the model dim (contraction dim of
   the QKV projections) lands on SBUF partitions.
 - Wq/Wk columns are permuted per head so RoPE pairs are de-interleaved to
   [real(64) | imag(64)]; scores are permutation-invariant since q and k are
   permuted identically. 1/sqrt(head_dim) is folded into Wq.
 - Projections produce qT/kT/vT [head_dim, S] directly (weights stationary,
   xT moving, N=512 => fp32r at full PE rate).
 - RoPE in T-layout: out = q*A + swap64(q*Bsw), where swap64 is a
   partition-half swap done with a tiny SBUF->SBUF DMA; A/Bsw are host-built
   [128, S] tables.
 - Attention is computed transposed: scoresT[k_row, q_row] = kT.T @ qT,
   exp on ScalarE (no max subtraction needed: |scores| <= ~9.3 by
   Cauchy-Schwarz on these magnitudes), bf16 probs.
 - o_unnormT[d, q_row] = sum_k v_tile[k,:].T @ expT (v in natural [k, d]
   bf16 layout via on-chip DMA transpose); row sums via a ones-column
   matmul; normalization deferred: oT * broadcast(1/rowsum) where the
   broadcast along partitions is a rank-1 matmul.
 - Final: out[q_row, :] = sum_h oT_h.T @ Wo_h with q_row on partitions.
"""

import os

import numpy as np
import ml_dtypes

import concourse.bass as bass
import concourse.bacc as bacc_mod
import concourse.mybir as mybir
import concourse.tile as tile
from concourse.bass_utils import run_bass_kernel_spmd

# Model constants (hardcoded per harness contract)
DIM = 2048
N_HEADS = 16
N_KV_HEADS = 4
HEAD_DIM = 128
N_REP = 4
SEQ = 2048
BATCH = 2

P = 128
KSUB = DIM // P          # 16 contraction subtiles for projections
NQH = N_REP              # 4 q heads per core
QD = NQH * HEAD_DIM      # 512 q dims per core
NQG = 4                  # 512-row groups per batch
QG = SEQ // NQG          # 512
SEQT = SEQ // P          # 16 seq tiles of 128

F32 = mybir.dt.float32
F32R = mybir.dt.float32r
BF16 = mybir.dt.bfloat16

LAST_RESULTS = None  # stash of BassKernelResults for test harness


def r(ap):
    return ap.bitcast(F32R)


def build_nc():
    nc = bacc_mod.Bacc("TRN2", target_bir_lowering=False)
    xT = nc.dram_tensor("xT", [DIM, SEQ], BF16, kind="ExternalInput")
    wq = nc.dram_tensor("wq", [DIM, QD], BF16, kind="ExternalInput")
    wkv = nc.dram_tensor("wkv", [DIM, 2 * HEAD_DIM], BF16, kind="ExternalInput")
    wo = nc.dram_tensor("wo", [QD, DIM], BF16, kind="ExternalInput")
    ropeA = nc.dram_tensor("ropeA", [P, SEQ], F32, kind="ExternalInput")
    ropeB = nc.dram_tensor("ropeB", [P, SEQ], F32, kind="ExternalInput")
    dmask = nc.dram_tensor("dmask", [P, 4 * QG], BF16, kind="ExternalInput")
    out = nc.dram_tensor("out", [SEQ, DIM], F32, kind="ExternalOutput")

    with tile.TileContext(nc) as tc:
        with (
            tc.tile_pool(name="consts", bufs=1) as consts,
            tc.tile_pool(name="xs", bufs=3) as xpool,
            tc.tile_pool(name="rope", bufs=2) as mpool,
            tc.tile_pool(name="exp", bufs=3) as epool,
            tc.tile_pool(name="norm", bufs=3) as npool,
            tc.tile_pool(name="outp", bufs=2) as opool,
        ):
            # ---- resident tensors (DMAs emitted close to first use) ----
            wq_sb = consts.tile([P, KSUB, QD], BF16)
            wkv_sb = consts.tile([P, KSUB, 2 * HEAD_DIM], BF16)
            wo_sb = consts.tile([P, NQH, DIM], BF16)
            A_sb = consts.tile([P, SEQ], F32)
            B_sb = consts.tile([P, SEQ], F32)
            dmask_sb = consts.tile([P, 4 * QG], BF16)
            ones_col = consts.tile([P, 1], BF16)
            nc.vector.memset(ones_col, 1.0)
            ones_row = consts.tile([1, P], F32)
            nc.vector.memset(ones_row, 1.0)

            qT_sb = consts.tile([P, NQH, SEQ], BF16)   # roped qT per head
            kT_sb = consts.tile([P, SEQ], BF16)       # roped kT
            vT_bf = consts.tile([P, SEQ], BF16)       # vT (staging)
            v_sb = consts.tile([P, SEQT, HEAD_DIM], BF16)  # v natural [krow, d]
            oT_sb = consts.tile([P, NQH, SEQ], BF16)  # normalized attn outT

            # weights per contraction subtile so the first matmuls only wait
            # on their own slice
            for k in range(KSUB):
                nc.sync.dma_start(wq_sb[:, k, :], wq[k * P:(k + 1) * P, :])
                nc.sync.dma_start(wkv_sb[:, k, :], wkv[k * P:(k + 1) * P, :])

            # ---- phase 1: QKV projections + rope, per 512-row group ----
            with tc.tile_pool(name="ps_proj", bufs=1, space="PSUM") as ps_proj:
                for qg in range(NQG):
                    rows = slice(qg * QG, (qg + 1) * QG)
                    # one psum tile per projection target -> fine-grained
                    # release so the next row-group's accumulation starts as
                    # soon as that target's rope is done
                    tgt_ps = [ps_proj.tile([P, QG], F32, tag=f"t{t}",
                                             name=f"tgt{t}_{qg}")
                              for t in range(6)]
                    for k in range(KSUB):
                        xt = xpool.tile([P, QG], BF16, tag="xt")
                        nc.sync.dma_start(xt, xT[k * P:(k + 1) * P, rows])
                        first, last = k == 0, k == KSUB - 1
                        for h in range(NQH):
                            nc.tensor.matmul(
                                tgt_ps[h],
                                wq_sb[:, k, h * P:(h + 1) * P],
                                xt, start=first, stop=last)
                        nc.tensor.matmul(tgt_ps[4], wkv_sb[:, k, 0:P],
                                         xt, start=first, stop=last)
                        nc.tensor.matmul(tgt_ps[5], wkv_sb[:, k, P:2 * P],
                                         xt, start=first, stop=last)
                    if qg == 0:
                        # rope tables + mask: needed from here on; emitted
                        # after the first matmul wave so they don't delay it
                        nc.sync.dma_start(A_sb, ropeA[:, :])
                        nc.sync.dma_start(B_sb, ropeB[:, :])
                        nc.sync.dma_start(dmask_sb, dmask[:, :])
                    # rope on 4 q heads + k
                    for t in range(5):
                        dst = qT_sb[:, t, rows] if t < NQH else kT_sb[:, rows]
                        m1 = mpool.tile([P, QG], F32, tag="m1")
                        m2 = mpool.tile([P, QG], F32, tag="m2")
                        m2s = mpool.tile([P, QG], F32, tag="m2s")
                        nc.vector.tensor_mul(m1, tgt_ps[t], A_sb[:, rows])
                        nc.vector.tensor_mul(m2, tgt_ps[t], B_sb[:, rows])
                        nc.sync.dma_start(m2s[0:64, :], m2[64:128, :])
                        nc.sync.dma_start(m2s[64:128, :], m2[0:64, :])
                        nc.vector.tensor_add(dst, m1, m2s)
                    # v: evacuate with bf16 cast, then transpose to natural
                    nc.vector.tensor_copy(vT_bf[:, rows], tgt_ps[5])
                    for j in range(QG // P):
                        kt = qg * (QG // P) + j
                        nc.sync.dma_start_transpose(
                            v_sb[:, kt, :], vT_bf[:, kt * P:(kt + 1) * P])

            # ---- phase 2: attention per (head, row-group), transposed ----
            with (
                tc.tile_pool(name="ps_sc", bufs=2, space="PSUM") as ps_sc,
                tc.tile_pool(name="ps_o", bufs=2, space="PSUM") as ps_o,
                tc.tile_pool(name="ps_rs", bufs=1, space="PSUM") as ps_rs,
                tc.tile_pool(name="ps_bc", bufs=1, space="PSUM") as ps_bc,
            ):
                # Wo weights: needed at phase 3; load during phase 2
                for h in range(NQH):
                    nc.sync.dma_start(wo_sb[:, h, :], wo[h * P:(h + 1) * P, :])
                for qg in range(NQG):
                    rows = slice(qg * QG, (qg + 1) * QG)
                    nkt = (qg + 1) * (QG // P)  # causal k tiles for this group
                    for h in range(NQH):
                        o_ps = ps_o.tile([P, QG], F32, tag="o")
                        rs_ps = ps_rs.tile([1, QG], F32, tag="rs")
                        ets = []
                        # pipeline: QK/exp emitted one group ahead of PV
                        for g in range(nkt // 2 + 1):
                            if g < nkt // 2:
                                sc_ps = ps_sc.tile([P, 2, QG], F32, tag="sc")
                                for j in range(2):
                                    kt = 2 * g + j
                                    nc.tensor.matmul(
                                        sc_ps[:, j, :],
                                        kT_sb[:, kt * P:(kt + 1) * P],
                                        qT_sb[:, h, rows],
                                        start=True, stop=True)
                                et = epool.tile([P, 2, QG], BF16, tag="et")
                                nc.scalar.activation(
                                    et, sc_ps,
                                    mybir.ActivationFunctionType.Exp)
                                if g >= 2 * qg:  # diagonal groups: mask
                                    m = g - 2 * qg
                                    nc.vector.tensor_mul(
                                        et, et,
                                        dmask_sb[:, 2 * m * QG:(2 * m + 2) * QG]
                                        .rearrange("p (a b) -> p a b", a=2))
                                ets.append(et)
                            if g > 0:  # PV + rowsum for the previous group
                                et = ets[g - 1]
                                for j in range(2):
                                    kt = 2 * (g - 1) + j
                                    first, last = kt == 0, kt == nkt - 1
                                    nc.tensor.matmul(
                                        o_ps, v_sb[:, kt, :], et[:, j, :],
                                        start=first, stop=last)
                                    nc.tensor.matmul(
                                        rs_ps, ones_col, et[:, j, :],
                                        start=first, stop=last)
                        # normalization: oT *= broadcast(1/rowsum)
                        rsr = npool.tile([1, QG], F32, tag="rsr")
                        nc.vector.reciprocal(rsr, rs_ps)
                        bc_ps = ps_bc.tile([P, QG], F32, tag="bc")
                        nc.tensor.matmul(bc_ps, ones_row, rsr,
                                         start=True, stop=True)
                        bc_sb = npool.tile([P, QG], F32, tag="bcs")
                        nc.vector.tensor_copy(bc_sb, bc_ps)
                        nc.vector.tensor_mul(oT_sb[:, h, rows], o_ps, bc_sb)

            # ---- phase 3: output projection per 128-row tile ----
            with tc.tile_pool(name="ps_wo", bufs=2, space="PSUM") as ps_wo:
                for qt in range(SEQT):
                    qsl = slice(qt * P, (qt + 1) * P)
                    wo_ps = ps_wo.tile([P, 4, QG], F32, tag="wo")
                    for h in range(NQH):  # stationary reused across n chunks
                        for n in range(4):
                            nc.tensor.matmul(
                                wo_ps[:, n, :],
                                oT_sb[:, h, qsl],
                                wo_sb[:, h, n * QG:(n + 1) * QG],
                                start=(h == 0), stop=(h == NQH - 1))
                    ot = opool.tile([P, 4, QG], F32, tag="ot")
                    nc.scalar.copy(ot, wo_ps)
                    nc.sync.dma_start(out[qsl, :], ot)
    nc.compile()
    return nc


_nc_cache = None


def _get_nc():
    global _nc_cache
    if _nc_cache is None:
        _nc_cache = build_nc()
    return _nc_cache


def _host_prep(x, freqs_cos, freqs_sin, Wq, Wk, Wv, Wo):
    x = np.asarray(x, dtype=np.float32)
    cos = np.asarray(freqs_cos, dtype=np.float32)
    sin = np.asarray(freqs_sin, dtype=np.float32)
    Wq = np.asarray(Wq, dtype=np.float32)
    Wk = np.asarray(Wk, dtype=np.float32)
    Wv = np.asarray(Wv, dtype=np.float32)
    Wo = np.asarray(Wo, dtype=np.float32)

    perm = np.concatenate([np.arange(0, HEAD_DIM, 2), np.arange(1, HEAD_DIM, 2)])
    scale = 1.0 / np.sqrt(np.float32(HEAD_DIM))
    Wq_p = (Wq.reshape(DIM, N_HEADS, HEAD_DIM)[:, :, perm] * scale).astype(np.float32)
    Wk_p = Wk.reshape(DIM, N_KV_HEADS, HEAD_DIM)[:, :, perm]

    # rope tables in T layout (partition = de-interleaved head dim)
    A = np.concatenate([cos.T, cos.T], axis=0).astype(np.float32)      # [128,S]
    Bsw = np.concatenate([sin.T, -sin.T], axis=0).astype(np.float32)   # [128,S]

    # diagonal causal masks: dmask[p, m*QG + q] = (p <= q - 128*m)
    pp = np.arange(P)[:, None]
    qq = np.arange(QG)[None, :]
    dm = np.concatenate([(pp <= qq - P * m) for m in range(4)], axis=1)
    dmask = dm.astype(ml_dtypes.bfloat16)

    xT = [np.ascontiguousarray(x[b].T).astype(ml_dtypes.bfloat16) for b in range(BATCH)]

    in_maps = []
    for core in range(8):
        b, g = divmod(core, N_KV_HEADS)
        wq_shard = np.ascontiguousarray(
            Wq_p[:, N_REP * g:N_REP * (g + 1), :].reshape(DIM, QD)
        ).astype(ml_dtypes.bfloat16)
        wkv_shard = np.ascontiguousarray(np.concatenate(
            [Wk_p[:, g, :], Wv[:, g * HEAD_DIM:(g + 1) * HEAD_DIM]],
            axis=1)).astype(ml_dtypes.bfloat16)
        wo_shard = np.ascontiguousarray(
            Wo[QD * g:QD * (g + 1), :]).astype(ml_dtypes.bfloat16)
        in_maps.append({
            "xT": xT[b],
            "wq": wq_shard,
            "wkv": wkv_shard,
            "wo": wo_shard,
            "ropeA": A,
            "ropeB": Bsw,
            "dmask": dmask,
        })
    return in_maps


def kernel(x, freqs_cos, freqs_sin, Wq, Wk, Wv, Wo):
    global LAST_RESULTS
    in_maps = _host_prep(x, freqs_cos, freqs_sin, Wq, Wk, Wv, Wo)
    nc = _get_nc()
    trace = bool(os.environ.get("KERNEL_TRACE"))
    res = run_bass_kernel_spmd(nc, in_maps, core_ids=list(range(8)), trace=trace)
    LAST_RESULTS = res
    outs = [m["out"] for m in res.results]
    out = np.stack(
        [sum(outs[b * N_KV_HEADS:(b + 1) * N_KV_HEADS]) for b in range(BATCH)],
        axis=0)
    return out.astype(np.float32)
